# revision 1
# baseline (speedup 1.0000x reference)
"""DeepFM (nn_DeepFM_33595234190062) Trainium2 Bass kernel.

Strategy: data-parallel over batch on 8 NeuronCores (2048 rows/core), the
343MB embedding tables replicated. Host packs emb1||emb0 into one
[F*V, 33] f32 table so a single 132B indirect-DMA descriptor fetches both
the latent vector and the linear weight for one (batch, field) lookup.

Per core: 16 batch tiles of 128 rows; per tile 26 indirect gathers
(one per field, 128 rows each), FM first/second-order terms on DVE,
PE transposes to feature-major, then a weight-stationary 3-layer MLP
(1248->256->128->64) kept feature-major throughout, and a fused head
producing both logits as a [2, 2048] output.
"""
import sys
import types
import numpy as np

B = 16384
F = 26
V = 100000
N = 13
D = 32
ROW = D + 1            # packed table row: 32 emb1 floats + 1 emb0 float
NCORES = 8
BC = B // NCORES       # batch rows per core
P = 128
NT = BC // P           # batch tiles per core (16)
NCHUNK = 4             # MLP batch chunks per core (512 cols each)
CHUNK = BC // NCHUNK
H1, H2, H3 = 256, 128, 64
K1 = (F + N) * D       # 1248


def _force_opt_stack():
    stale = [
        m for m in sys.modules
        if m.split(".")[0] in ("concourse", "bass_rust", "gauge", "gauge_rust")
        and getattr(sys.modules[m], "__file__", None)
        and "/_ro/" in (sys.modules[m].__file__ or "")
    ]
    for m in stale:
        del sys.modules[m]
    for p in ("/opt/pypackages", "/opt/trn_rl_repo"):
        if p in sys.path:
            sys.path.remove(p)
        sys.path.insert(0, p)


def _install_harness_patches():
    if "antenv.axon_hooks" not in sys.modules:
        mod = types.ModuleType("antenv.axon_hooks")
        mod._hook = None
        mod.set_axon_ntff_profile_hook = lambda h: setattr(mod, "_hook", h)
        mod.get_axon_ntff_profile_hook = lambda: mod._hook
        sys.modules["antenv.axon_hooks"] = mod
    try:
        from trn_agent_boot.trn_boot import _ntff_profile_via_ctypes
        sys.modules["antenv.axon_hooks"].set_axon_ntff_profile_hook(
            _ntff_profile_via_ctypes("/opt/axon/libaxon_pjrt.so")
        )
    except Exception:
        pass
    from concourse import bass_utils
    bass_utils.upload_artifacts = lambda tmpdir: f"local://{tmpdir}"


_force_opt_stack()
_install_harness_patches()

import concourse.bass as bass          # noqa: E402
import concourse.bacc as bacc          # noqa: E402
import concourse.tile as tile          # noqa: E402
from concourse import mybir            # noqa: E402
from concourse.masks import make_identity  # noqa: E402
from concourse.bass_utils import run_bass_kernel_spmd  # noqa: E402

AX = mybir.AxisListType
ALU = mybir.AluOpType
ACTF = mybir.ActivationFunctionType
F32 = mybir.dt.float32
I32 = mybir.dt.int32

_cache = {}


def _ap3(base, off, dims):
    """AP with explicit free dims; dims = [(step, num), ...]; keeps base partition dim."""
    return bass.AP(base.tensor, base.offset + off, [list(base.ap[0])] + [list(d) for d in dims])


def build_nc():
    nc = bacc.Bacc("TRN2", debug=False, num_devices=NCORES)

    tab = nc.dram_tensor("tab", [F * V, ROW], F32, kind="ExternalInput")
    idx = nc.dram_tensor("idx", [P, NT * F], I32, kind="ExternalInput")
    xnum = nc.dram_tensor("xnum", [BC, N], F32, kind="ExternalInput")
    nemb = nc.dram_tensor("nemb", [P, N * D], F32, kind="ExternalInput")
    nbias = nc.dram_tensor("nbias", [P, N], F32, kind="ExternalInput")
    w1 = nc.dram_tensor("w1", [K1, H1], F32, kind="ExternalInput")
    w2 = nc.dram_tensor("w2", [H1, H2], F32, kind="ExternalInput")
    w3 = nc.dram_tensor("w3", [H2, H3], F32, kind="ExternalInput")
    b1 = nc.dram_tensor("b1", [H1, 1], F32, kind="ExternalInput")
    b2 = nc.dram_tensor("b2", [H2, 1], F32, kind="ExternalInput")
    b3 = nc.dram_tensor("b3", [H3, 1], F32, kind="ExternalInput")
    wco = nc.dram_tensor("wco", [H3 + 2, 2], F32, kind="ExternalInput")
    bco = nc.dram_tensor("bco", [2, 1], F32, kind="ExternalInput")
    out = nc.dram_tensor("out", [2, BC], F32, kind="ExternalOutput")

    NK1 = 10  # k-tiles of 128 over 1248 (last = 96)

    with tile.TileContext(nc) as tc:
        with (
            tc.tile_pool(name="const", bufs=1) as cp,
            tc.tile_pool(name="g", bufs=3) as gp,
            tc.tile_pool(name="numt", bufs=3) as np_,
            tc.tile_pool(name="sq", bufs=2) as sqp,
            tc.tile_pool(name="fm", bufs=2) as fmp,
            tc.tile_pool(name="h0t", bufs=1) as h0p,
            tc.tile_pool(name="acts", bufs=1) as ap_,
            tc.tile_pool(name="w", bufs=1) as wp,
            tc.tile_pool(name="ptr", bufs=2, space="PSUM") as ptr,
            tc.tile_pool(name="pmm", bufs=2, space="PSUM") as pmm,
            tc.tile_pool(name="phd", bufs=2, space="PSUM") as phd,
        ):
            # ---- constants / weights ----
            ident = cp.tile([P, P], F32)
            make_identity(nc, ident[:])

            idx_t = cp.tile([P, NT * F], I32)
            nc.sync.dma_start(idx_t[:], idx[:, :])

            # replicate numeric embedding + bias across partitions
            nflat = cp.tile([P, N * D], F32)
            nc.sync.dma_start(nflat[:], nemb[:, :])
            nb_t = cp.tile([P, N], F32)
            nc.sync.dma_start(nb_t[:], nbias[:, :])

            w1_t = []
            for k in range(NK1):
                kp = min(128, K1 - k * 128)
                row = []
                for m in range(2):
                    t = wp.tile([P, P], F32, tag=f"w1_{k}_{m}")
                    nc.sync.dma_start(
                        t[:kp, :], w1[k * 128 : k * 128 + kp, m * 128 : (m + 1) * 128]
                    )
                    row.append(t)
                w1_t.append(row)
            w2_t = []
            for k in range(2):
                t = wp.tile([P, H2], F32, tag=f"w2_{k}")
                nc.sync.dma_start(t[:], w2[k * 128 : (k + 1) * 128, :])
                w2_t.append(t)
            w3_t = wp.tile([P, H3], F32, tag="w3")
            nc.sync.dma_start(w3_t[:], w3[:, :])
            wco_t = wp.tile([H3 + 2, 2], F32, tag="wco")
            nc.sync.dma_start(wco_t[:], wco[:, :])

            b1_t = [wp.tile([P, 1], F32, tag=f"b1_{m}", name=f"b1t{m}") for m in range(2)]
            for m in range(2):
                nc.sync.dma_start(b1_t[m][:], b1[m * 128 : (m + 1) * 128, :])
            b2_t = wp.tile([P, 1], F32, tag="b2")
            nc.sync.dma_start(b2_t[:], b2[:, :])
            b3_t = wp.tile([H3, 1], F32, tag="b3")
            nc.sync.dma_start(b3_t[:], b3[:, :])
            bco_t = wp.tile([2, 1], F32, tag="bco")
            nc.sync.dma_start(bco_t[:], bco[:, :])

            # ---- feature-major activations (full batch) ----
            h0T = [h0p.tile([P, BC], F32, tag=f"h0T_{k}", name=f"h0T{k}") for k in range(NK1)]
            h1T = [ap_.tile([P, BC], F32, tag=f"h1T_{m}", name=f"h1T{m}") for m in range(2)]
            h2T = ap_.tile([P, BC], F32, tag="h2T")
            catT = ap_.tile([H3 + 2, BC], F32, tag="catT")
            out_sb = ap_.tile([2, BC], F32, tag="out_sb")

            # ---- per-batch-tile: gather + FM + transposes ----
            for t in range(NT):
                g = gp.tile([P, F * ROW], F32)
                for f in range(F):
                    nc.gpsimd.indirect_dma_start(
                        out=g[:, f * ROW : (f + 1) * ROW],
                        out_offset=None,
                        in_=tab[:, :],
                        in_offset=bass.IndirectOffsetOnAxis(
                            ap=idx_t[:, t * F + f : t * F + f + 1], axis=0
                        ),
                    )

                xn = np_.tile([P, N], F32, tag="xn")
                nc.sync.dma_start(xn[:], xnum[t * P : (t + 1) * P, :])

                # numeric embeddings: num[:, f*32:(f+1)*32] = nflat_f * x_num[:, f]
                num = np_.tile([P, N * D], F32, tag="num")
                for f in range(N):
                    nc.vector.tensor_scalar(
                        out=num[:, f * D : (f + 1) * D],
                        in0=nflat[:, f * D : (f + 1) * D],
                        scalar1=xn[:, f : f + 1],
                        scalar2=None,
                        op0=ALU.mult,
                    )

                gap = g[:]
                nap = num[:]
                # compact cat embeddings to contiguous [P, 832] for PE transposes
                h0b = gp.tile([P, F * D], F32, tag="h0b")
                nc.vector.tensor_copy(
                    out=_ap3(h0b[:], 0, [(D, F), (1, D)]),
                    in_=_ap3(gap, 0, [(ROW, F), (1, D)]),
                )
                # S = sum_f v_f   [P, D]
                S = fmp.tile([P, D], F32, tag="S")
                Sn = fmp.tile([P, D], F32, tag="Sn")
                nc.vector.tensor_reduce(
                    out=S[:], in_=_ap3(gap, 0, [(1, D), (ROW, F)]),
                    axis=AX.X, op=ALU.add,
                )
                nc.vector.tensor_reduce(
                    out=Sn[:], in_=_ap3(nap, 0, [(1, D), (D, N)]),
                    axis=AX.X, op=ALU.add,
                )
                nc.vector.tensor_tensor(out=S[:], in0=S[:], in1=Sn[:], op=ALU.add)

                # sqsum = sum_{f,d} v^2  [P, 1]
                sqc = sqp.tile([P, F * ROW], F32, tag="sqc")
                nc.vector.tensor_tensor(out=sqc[:], in0=gap, in1=gap, op=ALU.mult)
                sqn = sqp.tile([P, N * D], F32, tag="sqn")
                nc.vector.tensor_tensor(out=sqn[:], in0=nap, in1=nap, op=ALU.mult)
                qs = fmp.tile([P, 1], F32, tag="qs")
                qs2 = fmp.tile([P, 1], F32, tag="qs2")
                nc.vector.tensor_reduce(
                    out=qs[:], in_=_ap3(sqc[:], 0, [(ROW, F), (1, D)]),
                    axis=AX.XY, op=ALU.add,
                )
                nc.vector.tensor_reduce(
                    out=qs2[:], in_=_ap3(sqn[:], 0, [(D, N), (1, D)]),
                    axis=AX.XY, op=ALU.add,
                )
                nc.vector.tensor_tensor(out=qs[:], in0=qs[:], in1=qs2[:], op=ALU.add)

                # ||S||^2  [P, 1]
                ssq = fmp.tile([P, D], F32, tag="ssq")
                nc.vector.tensor_tensor(out=ssq[:], in0=S[:], in1=S[:], op=ALU.mult)
                s2 = fmp.tile([P, 1], F32, tag="s2")
                nc.vector.tensor_reduce(out=s2[:], in_=ssq[:], axis=AX.X, op=ALU.add)

                # lf[:,0] = linear term, lf[:,1] = fm term
                lf = fmp.tile([P, 2], F32, tag="lf")
                l0 = fmp.tile([P, 1], F32, tag="l0")
                nc.vector.tensor_reduce(
                    out=l0[:], in_=_ap3(gap, D, [(ROW, F)]), axis=AX.X, op=ALU.add
                )
                xnb = fmp.tile([P, N], F32, tag="xnb")
                nc.vector.tensor_tensor(out=xnb[:], in0=xn[:], in1=nb_t[:], op=ALU.mult)
                l1 = fmp.tile([P, 1], F32, tag="l1")
                nc.vector.tensor_reduce(out=l1[:], in_=xnb[:], axis=AX.X, op=ALU.add)
                nc.vector.tensor_tensor(out=lf[:, 0:1], in0=l0[:], in1=l1[:], op=ALU.add)
                fmv = fmp.tile([P, 1], F32, tag="fmv")
                nc.vector.tensor_tensor(out=fmv[:], in0=s2[:], in1=qs[:], op=ALU.subtract)
                nc.scalar.activation(
                    out=lf[:, 1:2], in_=fmv[:], func=ACTF.Copy, scale=0.5
                )

                # transpose lf -> catT[64:66, t*128:(t+1)*128]
                plf = ptr.tile([2, P], F32, tag="pt")
                nc.tensor.transpose(out=plf[:], in_=lf[:], identity=ident[:])
                nc.scalar.activation(
                    out=catT[H3 : H3 + 2, t * P : (t + 1) * P],
                    in_=plf[:], func=ACTF.Copy,
                )

                # transposes to feature-major h0T
                # k 0..5: cat fields 4f..4f+3 (strided 33)
                for k in range(6):
                    pt = ptr.tile([P, P], F32, tag="pt")
                    nc.tensor.transpose(
                        out=pt[:],
                        in_=h0b[:, k * 128 : (k + 1) * 128],
                        identity=ident[:],
                    )
                    nc.scalar.activation(
                        out=h0T[k][:, t * P : (t + 1) * P], in_=pt[:], func=ACTF.Copy
                    )
                # k 6: cat fields 24,25 (64) + num fields 0,1 (64)
                pt6a = ptr.tile([64, P], F32, tag="pt")
                nc.tensor.transpose(
                    out=pt6a[:], in_=h0b[:, 768:832], identity=ident[:]
                )
                nc.scalar.activation(
                    out=h0T[6][0:64, t * P : (t + 1) * P], in_=pt6a[:], func=ACTF.Copy
                )
                pt6b = ptr.tile([64, P], F32, tag="pt")
                nc.tensor.transpose(
                    out=pt6b[:], in_=num[:, 0 : 2 * D], identity=ident[:]
                )
                nc.scalar.activation(
                    out=h0T[6][64:128, t * P : (t + 1) * P], in_=pt6b[:], func=ACTF.Copy
                )
                # k 7,8: num fields 2..5, 6..9 ; k 9: num fields 10..12 (96)
                for j, (lo, sz) in enumerate([(2, 128), (6, 128), (10, 96)]):
                    pt = ptr.tile([P, P], F32, tag="pt")
                    nc.tensor.transpose(
                        out=pt[0:sz, :],
                        in_=num[:, lo * D : lo * D + sz],
                        identity=ident[:],
                    )
                    nc.scalar.activation(
                        out=h0T[7 + j][:sz, t * P : (t + 1) * P],
                        in_=pt[:sz, :], func=ACTF.Copy,
                    )

            # ---- MLP, feature-major, per 512-column chunk ----
            for c in range(NCHUNK):
                cs = slice(c * CHUNK, (c + 1) * CHUNK)
                # layer 1: h1T[m] = relu(W1[:,m].T @ h0T + b1[m])
                for m in range(2):
                    pm = pmm.tile([P, CHUNK], F32, tag="pm")
                    for k in range(NK1):
                        kp = min(128, K1 - k * 128)
                        nc.tensor.matmul(
                            out=pm[:],
                            lhsT=w1_t[k][m][:kp, :],
                            rhs=h0T[k][:kp, cs],
                            start=(k == 0),
                            stop=(k == NK1 - 1),
                        )
                    nc.scalar.activation(
                        out=h1T[m][:, cs], in_=pm[:], func=ACTF.Relu, bias=b1_t[m][:]
                    )
                # layer 2
                pm2 = pmm.tile([P, CHUNK], F32, tag="pm")
                for k in range(2):
                    nc.tensor.matmul(
                        out=pm2[:], lhsT=w2_t[k][:], rhs=h1T[k][:, cs],
                        start=(k == 0), stop=(k == 1),
                    )
                nc.scalar.activation(
                    out=h2T[:, cs], in_=pm2[:], func=ACTF.Relu, bias=b2_t[:]
                )
                # layer 3
                pm3 = pmm.tile([P, CHUNK], F32, tag="pm")
                nc.tensor.matmul(
                    out=pm3[:H3, :], lhsT=w3_t[:], rhs=h2T[:, cs], start=True, stop=True
                )
                nc.scalar.activation(
                    out=catT[:H3, cs], in_=pm3[:H3, :], func=ACTF.Relu, bias=b3_t[:]
                )
                # head: [2, CHUNK] = wco.T @ catT + bco
                ph = phd.tile([2, CHUNK], F32, tag="ph")
                nc.tensor.matmul(
                    out=ph[:], lhsT=wco_t[:], rhs=catT[:, cs], start=True, stop=True
                )
                nc.scalar.activation(
                    out=out_sb[:, cs], in_=ph[:], func=ACTF.Identity, bias=bco_t[:]
                )

            nc.sync.dma_start(out[:, :], out_sb[:])

    nc.compile()
    return nc


def _prepare(x_cat, x_num, emb0, emb1, num_bias, num_emb,
             W1, b1, W2, b2, W3, b3, Wc, bc, Wo, bo):
    x_cat = np.asarray(x_cat)
    x_num = np.asarray(x_num, dtype=np.float32)
    emb0 = np.asarray(emb0, dtype=np.float32)
    emb1 = np.asarray(emb1, dtype=np.float32)

    if "nc" not in _cache:
        _cache["nc"] = build_nc()
    nc = _cache["nc"]

    # packed table [F*V, 33]
    tab = np.empty((F * V, ROW), dtype=np.float32)
    tab[:, :D] = emb1.reshape(F * V, D)
    tab[:, D] = emb0.reshape(F * V)

    flat = (np.arange(F, dtype=np.int64)[None, :] * V + x_cat).astype(np.int32)  # [B, F]

    common = dict(
        tab=tab,
        nemb=np.broadcast_to(np.asarray(num_emb, np.float32).reshape(1, N * D), (P, N * D)).copy(),
        nbias=np.broadcast_to(np.asarray(num_bias, np.float32).reshape(1, N), (P, N)).copy(),
        w1=np.asarray(W1, np.float32), w2=np.asarray(W2, np.float32),
        w3=np.asarray(W3, np.float32),
        b1=np.asarray(b1, np.float32).reshape(H1, 1),
        b2=np.asarray(b2, np.float32).reshape(H2, 1),
        b3=np.asarray(b3, np.float32).reshape(H3, 1),
        wco=np.concatenate(
            [np.asarray(Wc, np.float32), np.asarray(Wo, np.float32)], axis=1
        ),
        bco=np.array(
            [[np.float32(np.asarray(bc).reshape(()))],
             [np.float32(np.asarray(bo).reshape(()))]], dtype=np.float32
        ),
    )

    in_maps = []
    for c in range(NCORES):
        fc = flat[c * BC : (c + 1) * BC]          # [BC, F]
        # idx[p, t*F + f] = fc[t*128 + p, f]
        idxc = np.ascontiguousarray(
            fc.reshape(NT, P, F).transpose(1, 0, 2).reshape(P, NT * F)
        )
        in_maps.append(dict(common, idx=idxc, xnum=x_num[c * BC : (c + 1) * BC]))

    return nc, in_maps


def _collect(res):
    outs = [res.results[c]["out"] for c in range(NCORES)]   # each [2, BC]
    full = np.concatenate(outs, axis=1)                     # [2, B]
    logit1 = full[0].reshape(B, 1).astype(np.float32)
    logit2 = full[1].reshape(B, 1).astype(np.float32)
    return logit1, logit2


def kernel(**inputs):
    nc, in_maps = _prepare(**inputs)
    res = run_bass_kernel_spmd(nc, in_maps, core_ids=list(range(NCORES)))
    return _collect(res)


def run_traced(**inputs):
    """Like kernel() but with NTFF profiling; returns BassKernelResults."""
    nc, in_maps = _prepare(**inputs)
    return run_bass_kernel_spmd(
        nc, in_maps, core_ids=list(range(NCORES)), trace=True
    )

